# revision 16
# baseline (speedup 1.0000x reference)
"""NeuralMemory (Titans-style) TRN2 kernel.

Sharding: 8 cores = (batch b in {0,1}) x (head h in {0..3}). Each core runs the
full store->scan->retrieve pipeline for one (b, h) pair on its 2048 tokens and
produces a partial output projection; the host sums the 4 head partials per
batch and adds b_comb.
"""
import numpy as np

import concourse.bacc as bacc
import concourse.tile as tile
import concourse.mybir as mybir
from concourse import bass_utils

f32 = mybir.dt.float32
f32r = mybir.dt.float32r
bf16 = mybir.dt.bfloat16
AF = mybir.ActivationFunctionType
OP = mybir.AluOpType
AX = mybir.AxisListType

DIM = 512
HEADS = 4
DH = 128
HID = 512
CHUNK = 64
NCH = 32
N = 2048
NT = 16          # 128-token tiles
B = 2
MAX_LR = 0.01
EPS = 1e-6
PCOLS = 392      # projection output cols: k(128) v(128) q(128) lr gate mom dec pad(4)

_CACHE = {}


def ts(i, sz):
    return slice(i * sz, (i + 1) * sz)


def _build():
    nc = bacc.Bacc("TRN2", target_bir_lowering=False, debug=False)

    dt_in = {}

    def dram(name, shape, dt, kind="ExternalInput"):
        dt_in[name] = (shape, dt)
        return nc.dram_tensor(name, list(shape), dt, kind=kind).ap()

    x_d = dram("x", (N, DIM), f32)
    projw_d = dram("projw", (4, 128, PCOLS), f32r)
    w1f_d = dram("w1f", (128, HID), f32)
    w1b_d = dram("w1b", (128, HID), bf16)
    w2nf_d = dram("w2nf", (128, 4, 128), f32)
    w2nb_d = dram("w2nb", (128, 4, 128), bf16)
    w2tb_d = dram("w2tb", (128, HID), bf16)
    wcombb_d = dram("wcombb", (128, DIM), bf16)
    gbrow_d = dram("gbrow", (1, 256), f32r)
    gbcol_d = dram("gbcol", (128, 2), f32)
    identf_d = dram("identf", (128, 128), f32)
    identb_d = dram("identb", (128, 128), bf16)
    ones1_d = dram("ones1", (1, 128), f32r)
    onescol_d = dram("onescol", (128, 128), f32r)   # col0 = ones, rest 0
    mask2_d = dram("mask2", (128, 2), f32)
    maskmean_d = dram("maskmean", (128, 2), f32)
    biaslg_d = dram("biaslg", (1, 2), f32)
    biasmd_d = dram("biasmd", (1, 2 * NCH), f32)
    out_d = dram("out", (N, DIM), f32, kind="ExternalOutput")

    with tile.TileContext(nc) as tc:
        with tc.tile_pool(name="persist", bufs=1) as pp, \
             tc.tile_pool(name="work", bufs=3) as wk:

            # ---------------- setup: load weights/constants ----------------
            projw = pp.tile([128, 4, PCOLS], f32r)
            nc.sync.dma_start(projw, projw_d.rearrange("j p c -> p j c"))
            w1f = pp.tile([128, HID], f32)
            nc.sync.dma_start(w1f, w1f_d)
            w1b = pp.tile([128, HID], bf16)
            nc.sync.dma_start(w1b, w1b_d)
            w2nf = pp.tile([128, 4, 128], f32)
            nc.sync.dma_start(w2nf, w2nf_d)
            w2nb = pp.tile([128, 4, 128], bf16)
            nc.sync.dma_start(w2nb, w2nb_d)
            w2tb = pp.tile([128, HID], bf16)
            nc.sync.dma_start(w2tb, w2tb_d)
            wcombb = pp.tile([128, DIM], bf16)
            nc.sync.dma_start(wcombb, wcombb_d)
            gbrow = pp.tile([1, 256], f32r)
            nc.sync.dma_start(gbrow, gbrow_d)
            identf = pp.tile([128, 128], f32)
            nc.sync.dma_start(identf, identf_d)
            identb = pp.tile([128, 128], bf16)
            nc.sync.dma_start(identb, identb_d)
            ones1 = pp.tile([1, 128], f32r)
            nc.sync.dma_start(ones1, ones1_d)
            onescol = pp.tile([128, 128], f32r)
            nc.sync.dma_start(onescol, onescol_d)
            mask2 = pp.tile([128, 2], f32)
            nc.sync.dma_start(mask2, mask2_d)
            maskmean = pp.tile([128, 2], f32)
            nc.sync.dma_start(maskmean, maskmean_d)
            biaslg = pp.tile([128, 2], f32)
            nc.sync.dma_start(biaslg, biaslg_d.to_broadcast((128, 2)))
            biasmd = pp.tile([1, 2 * NCH], f32)
            nc.sync.dma_start(biasmd, biasmd_d)

            # running scan state
            w1c = pp.tile([128, HID], f32)
            nc.vector.tensor_copy(w1c, w1f)
            w2c = pp.tile([128, 4, 128], f32)
            nc.vector.tensor_copy(w2c, w2nf)
            w1cb = pp.tile([128, HID], bf16)
            nc.gpsimd.tensor_copy(w1cb, w1f)
            w2cb = pp.tile([128, 4, 128], bf16)
            nc.gpsimd.tensor_copy(w2cb, w2nf)
            m1 = pp.tile([128, HID], f32)
            nc.vector.memset(m1, 0.0)
            m2 = pp.tile([128, 4, 128], f32)
            nc.vector.memset(m2, 0.0)
            gbc = pp.tile([128, 2], f32)
            nc.sync.dma_start(gbc, gbcol_d)
            mgb = pp.tile([128, 2], f32)
            nc.vector.memset(mgb, 0.0)

            epsln = pp.tile([128, 1], f32)
            nc.vector.memset(epsln, EPS)
            eps1a = pp.tile([1, 1], f32)
            nc.vector.memset(eps1a, EPS)
            eps12 = pp.tile([128, 1], f32)
            nc.vector.memset(eps12, 1e-12)

            # persistent activation storage
            k_sb = pp.tile([128, NT, 128], f32)
            kb_sb = pp.tile([128, NT, 128], bf16)
            q_sb = pp.tile([128, NT, 128], f32)
            v_sb = pp.tile([128, NT, 128], f32)
            kTb = pp.tile([128, N], bf16)
            qTf = pp.tile([128, N], f32)
            qTb = pp.tile([128, N], bf16)
            xss = pp.tile([128, NT], f32)
            kss = pp.tile([128, NT], f32)
            qss = pp.tile([128, NT], f32)
            rstd = pp.tile([128, NT], f32)
            combk = pp.tile([128, NT], f32)
            combq = pp.tile([128, NT], f32)
            slr = pp.tile([128, NT], f32)
            gatec = pp.tile([128, NT], f32)
            zmd = pp.tile([128, NT, 2], f32)
            grep = pp.tile([128, 128], f32)
            brep = pp.tile([128, 128], f32)
            mdrep = pp.tile([128, 2 * NCH], f32)
            murstd = pp.tile([1, 256], f32r)
            ysq = pp.tile([128, 256], f32r)
            gbsnap = pp.tile([128, 4], f32)
            sgb = pp.tile([128, 4], f32)

            # ---------------- phase 1a: load x, project ----------------
            with tc.tile_pool(name="psT", bufs=4, space="PSUM") as psT, \
                 tc.tile_pool(name="psP", bufs=2, space="PSUM") as psP:

                for t in range(NT):
                    x_t = wk.tile([128, DIM], f32)
                    nc.sync.dma_start(x_t, x_d[ts(t, 128), :])
                    # per-token sum(x^2) via ACT Square w/ accumulate
                    sq = wk.tile([128, DIM], f32)
                    nc.scalar.activation(sq, x_t, AF.Square,
                                         accum_out=xss[:, t:t + 1])
                    # transpose x tile -> xT (f32r rounding producer = copy)
                    xT = wk.tile([128, 4, 128], f32r)
                    for j in range(4):
                        pt = psT.tile([128, 128], f32, tag="pt")
                        nc.tensor.transpose(pt, x_t[:, ts(j, 128)], identf)
                        if j % 2 == 0:
                            nc.scalar.copy(xT[:, j, :], pt)
                        else:
                            nc.vector.tensor_copy(xT[:, j, :], pt)
                    # fused projection: [tok,392] = x_tile @ projW
                    ppj = psP.tile([128, PCOLS], f32, tag="ppj")
                    for j in range(4):
                        nc.tensor.matmul(ppj, xT[:, j, :], projw[:, j, :],
                                         start=(j == 0), stop=(j == 3))
                    # stash raw k, v, q + sumsq of k,q
                    nc.vector.tensor_copy(k_sb[:, t, :], ppj[:, 0:128])
                    nc.scalar.copy(v_sb[:, t, :], ppj[:, 128:256])
                    nc.vector.tensor_copy(q_sb[:, t, :], ppj[:, 256:384])
                    sqk = wk.tile([128, 128], f32)
                    nc.scalar.activation(sqk, ppj[:, 0:128], AF.Square,
                                         accum_out=kss[:, t:t + 1])
                    sqq = wk.tile([128, 128], f32)
                    nc.scalar.activation(sqq, ppj[:, 256:384], AF.Square,
                                         accum_out=qss[:, t:t + 1])
                    # scalar projection cols (raw; scaled in phase 1b)
                    nc.vector.tensor_copy(zmd[:, t, :], ppj[:, 386:388])
                    nc.vector.tensor_copy(slr[:, t:t + 1], ppj[:, 384:385])
                    nc.vector.tensor_copy(gatec[:, t:t + 1], ppj[:, 385:386])

                # ---------------- phase 1b: batched scalars ----------------
                # rstd = 1/sqrt(mean(x^2)+eps)
                tmpA = wk.tile([128, NT], f32, tag="tmpA")
                nc.scalar.activation(tmpA, xss, AF.Sqrt, bias=epsln,
                                     scale=1.0 / DIM)
                nc.vector.reciprocal(rstd, tmpA)
                # comb_k = rstd / sqrt(rstd^2*kss + 1e-12)
                for ss, comb in ((kss, combk), (qss, combq)):
                    t1 = wk.tile([128, NT], f32, tag="t1")
                    nc.vector.tensor_tensor(t1, rstd, rstd, op=OP.mult)
                    nc.vector.tensor_tensor(t1, t1, ss, op=OP.mult)
                    t2 = wk.tile([128, NT], f32, tag="t2")
                    nc.scalar.activation(t2, t1, AF.Sqrt, bias=eps12)
                    nc.vector.reciprocal(t2, t2)
                    nc.vector.tensor_tensor(comb, t2, rstd, op=OP.mult)
                # lr / gate: sigmoid(z*rstd + bias)
                nc.vector.tensor_tensor(slr, slr, rstd, op=OP.mult)
                nc.scalar.activation(slr, slr, AF.Sigmoid, bias=biaslg[:, 0:1])
                nc.vector.tensor_scalar(slr, slr, 2.0 * MAX_LR / DH, None,
                                        op0=OP.mult)
                nc.vector.tensor_tensor(gatec, gatec, rstd, op=OP.mult)
                nc.scalar.activation(gatec, gatec, AF.Sigmoid,
                                     bias=biaslg[:, 1:2])
                # mom/dec: pooled per chunk then sigmoid
                nc.vector.tensor_tensor(zmd[:, :, 0], zmd[:, :, 0], rstd,
                                        op=OP.mult)
                nc.vector.tensor_tensor(zmd[:, :, 1], zmd[:, :, 1], rstd,
                                        op=OP.mult)
                psmd = psP.tile([1, 2 * NCH], f32, tag="psmd")
                for t in range(NT):
                    nc.tensor.matmul(psmd[:, ts(t, 2)], zmd[:, t, 0:1],
                                     maskmean, start=True, stop=True)
                    nc.tensor.matmul(psmd[:, NCH + 2 * t:NCH + 2 * t + 2],
                                     zmd[:, t, 1:2], maskmean, start=True,
                                     stop=True)
                mdf = wk.tile([1, 2 * NCH], f32, tag="mdf")
                nc.vector.tensor_tensor(mdf, psmd, biasmd, op=OP.add)
                nc.scalar.activation(mdf, mdf, AF.Sigmoid)
                mdrow = pp.tile([1, 2 * NCH], f32r)
                nc.vector.tensor_copy(mdrow[:, 0:NCH], mdf[:, 0:NCH])
                # second half becomes (1 - dec)
                nc.vector.tensor_scalar(mdrow[:, NCH:2 * NCH],
                                        mdf[:, NCH:2 * NCH], -1.0, 1.0,
                                        op0=OP.mult, op1=OP.add)
                pmd = psT.tile([128, 128], f32, tag="pt")
                nc.tensor.matmul(pmd[:, 0:2 * NCH], ones1, mdrow,
                                 start=True, stop=True)
                nc.vector.tensor_copy(mdrep, pmd[:, 0:2 * NCH])
                # g/b replicated token-major
                pgb = psT.tile([128, 128], f32, tag="pt")
                nc.tensor.matmul(pgb, ones1, gbrow[0:1, 0:128], start=True,
                                 stop=True)
                nc.vector.tensor_copy(grep, pgb)
                pgb2 = psT.tile([128, 128], f32, tag="pt")
                nc.tensor.matmul(pgb2, ones1, gbrow[0:1, 128:256], start=True,
                                 stop=True)
                nc.vector.tensor_copy(brep, pgb2)

                # ---------------- phase 1c: normalize + kT/qT ----------------
                for t in range(NT):
                    nc.vector.tensor_scalar(k_sb[:, t, :], k_sb[:, t, :],
                                            combk[:, t:t + 1], None,
                                            op0=OP.mult)
                    nc.vector.tensor_scalar(q_sb[:, t, :], q_sb[:, t, :],
                                            combq[:, t:t + 1], None,
                                            op0=OP.mult)
                    pk = psT.tile([128, 128], f32, tag="pt")
                    nc.tensor.transpose(pk, k_sb[:, t, :], identf)
                    nc.vector.tensor_copy(kTb[:, ts(t, 128)], pk)
                    pq = psT.tile([128, 128], f32, tag="pt")
                    nc.tensor.transpose(pq, q_sb[:, t, :], identf)
                    nc.scalar.copy(qTf[:, ts(t, 128)], pq)
                nc.gpsimd.tensor_copy(kb_sb, k_sb)
                nc.gpsimd.tensor_copy(qTb, qTf)

            # ---------------- phase 2: grads + scan + retrieve ----------------
            with tc.tile_pool(name="psA", bufs=3, space="PSUM") as psA, \
                 tc.tile_pool(name="psW", bufs=3, space="PSUM") as psW, \
                 tc.tile_pool(name="psR", bufs=2, space="PSUM") as psR:

                for t in range(NT):
                    # ---- gradient phase (both chunks of tile t batched) ----
                    ph1T = psA.tile([128, HID], f32, tag="a")
                    for j in range(4):
                        nc.tensor.matmul(ph1T[:, ts(j, 128)], w1b[:, ts(j, 128)],
                                         kTb[:, ts(t, 128)], start=True,
                                         stop=True)
                    hgTb = wk.tile([128, 4, 128], bf16, tag="hgTb")
                    nc.scalar.activation(hgTb, ph1T, AF.Gelu)
                    ph1 = psA.tile([128, HID], f32, tag="a")
                    nc.tensor.matmul(ph1, kTb[:, ts(t, 128)], w1b, start=True,
                                     stop=True)
                    hgb = wk.tile([128, HID], bf16, tag="hgb")
                    nc.scalar.activation(hgb, ph1, AF.Gelu)
                    gdb = wk.tile([128, HID], bf16, tag="gdb")
                    nc.scalar.activation(gdb, ph1, AF.Derivative_Gelu)
                    py2 = psA.tile([128, 128], f32, tag="a")
                    for j in range(4):
                        nc.tensor.matmul(py2, hgTb[:, j, :], w2nb[:, j, :],
                                         start=(j == 0), stop=(j == 3))
                    y_sb = wk.tile([128, 128], f32, tag="y_sb")
                    nc.vector.tensor_tensor(y_sb, py2, k_sb[:, t, :], op=OP.add)
                    # LN forward
                    st6 = wk.tile([128, 6], f32, tag="st6")
                    nc.vector.bn_stats(st6, y_sb)
                    mv = wk.tile([128, 2], f32, tag="mv")
                    nc.vector.bn_aggr(mv, st6)
                    sd = wk.tile([128, 1], f32, tag="sd")
                    nc.scalar.activation(sd, mv[:, 1:2], AF.Sqrt, bias=epsln)
                    rstdln = wk.tile([128, 1], f32, tag="rstdln")
                    nc.vector.reciprocal(rstdln, sd)
                    xhat = wk.tile([128, 128], f32, tag="xhat")
                    nc.vector.tensor_scalar(xhat, y_sb, mv[:, 0:1], rstdln,
                                            op0=OP.subtract, op1=OP.mult)
                    pred = wk.tile([128, 128], f32, tag="pred")
                    nc.gpsimd.tensor_tensor(pred, xhat, grep, op=OP.mult)
                    nc.gpsimd.tensor_tensor(pred, pred, brep, op=OP.add)
                    # dpred (negated grad): (v*rstd - pred) * (2*lr/DH)
                    dpred = wk.tile([128, 128], f32, tag="dpred")
                    nc.vector.scalar_tensor_tensor(dpred, in0=v_sb[:, t, :],
                                                   scalar=rstd[:, t:t + 1],
                                                   in1=pred, op0=OP.mult,
                                                   op1=OP.subtract)
                    nc.vector.tensor_scalar(dpred, dpred, slr[:, t:t + 1], None,
                                            op0=OP.mult)
                    # dg/db chunk sums
                    e_sb = wk.tile([128, 128], f32, tag="e_sb")
                    nc.gpsimd.tensor_tensor(e_sb, dpred, xhat, op=OP.mult)
                    pgb_ps = psW.tile([128, 4], f32, tag="w")
                    nc.tensor.matmul(pgb_ps[:, 0:2], e_sb, mask2, start=True,
                                     stop=True)
                    nc.tensor.matmul(pgb_ps[:, 2:4], dpred, mask2, start=True,
                                     stop=True)
                    nc.vector.tensor_copy(sgb, pgb_ps)
                    # LN backward -> dy (bf16)
                    dxh = wk.tile([128, 128], f32, tag="dxh")
                    nc.vector.tensor_tensor(dxh, dpred, grep, op=OP.mult)
                    r1 = wk.tile([128, 1], f32, tag="r1")
                    nc.vector.tensor_reduce(r1, dxh, axis=AX.X, op=OP.add)
                    u_sb = wk.tile([128, 128], f32, tag="u_sb")
                    nc.gpsimd.tensor_tensor(u_sb, dxh, xhat, op=OP.mult)
                    r2 = wk.tile([128, 1], f32, tag="r2")
                    nc.vector.tensor_reduce(r2, u_sb, axis=AX.X, op=OP.add)
                    nc.vector.tensor_scalar(r1, r1, rstdln, 1.0 / DH,
                                            op0=OP.mult, op1=OP.mult)
                    nc.vector.tensor_scalar(r2, r2, rstdln, -1.0 / DH,
                                            op0=OP.mult, op1=OP.mult)
                    a_sb = wk.tile([128, 128], f32, tag="a_sb")
                    nc.vector.tensor_scalar(a_sb, dxh, rstdln, r1,
                                            op0=OP.mult, op1=OP.subtract)
                    dyb = wk.tile([128, 128], bf16, tag="dyb")
                    nc.vector.scalar_tensor_tensor(dyb, in0=xhat, scalar=r2,
                                                   in1=a_sb, op0=OP.mult,
                                                   op1=OP.add)
                    # dh1 = (dy @ w2T) * gelu'(h1)
                    pdyT = psA.tile([128, 128], bf16, tag="a")
                    nc.tensor.transpose(pdyT, dyb, identb)
                    dyTb = wk.tile([128, 128], bf16, tag="dyTb")
                    nc.vector.tensor_copy(dyTb, pdyT)
                    pdh1 = psA.tile([128, HID], f32, tag="a")
                    nc.tensor.matmul(pdh1, dyTb, w2tb, start=True, stop=True)
                    dh1b = wk.tile([128, HID], bf16, tag="dh1b")
                    nc.vector.tensor_tensor(dh1b, pdh1, gdb, op=OP.mult)

                    # ---- per chunk: dw, retrieve, scan update ----
                    for cl in range(2):
                        c = 2 * t + cl
                        prt = slice(64 * cl, 64 * cl + 64)
                        # dw mms for this chunk
                        pdw1 = psW.tile([128, HID], f32, tag="w")
                        nc.tensor.matmul(pdw1, kb_sb[prt, t, :], dh1b[prt, :],
                                         start=True, stop=True)
                        pdw2 = psW.tile([128, 4, 128], f32, tag="w")
                        for j in range(4):
                            nc.tensor.matmul(pdw2[:, j, :],
                                             hgb[prt, ts(j, 128)], dyb[prt, :],
                                             start=True, stop=True)
                        # retrieve chunk c with current weights
                        prh1 = psR.tile([128, 4, 64], f32, tag="r")
                        for j in range(4):
                            nc.tensor.matmul(prh1[:, j, :], w1cb[:, ts(j, 128)],
                                             qTb[:, ts(c, 64)], start=True,
                                             stop=True)
                        hgrb = wk.tile([128, 4, 64], bf16, tag="hgrb")
                        nc.scalar.activation(hgrb, prh1, AF.Gelu)
                        pry2 = psR.tile([128, 64], f32, tag="r")
                        for j in range(4):
                            nc.tensor.matmul(pry2, w2cb[:, j, :], hgrb[:, j, :],
                                             start=(j == 0), stop=(j == 3))
                        nc.vector.tensor_tensor(ysq[:, ts(cl, 64)], pry2,
                                                qTf[:, ts(c, 64)], op=OP.add)
                        nc.vector.tensor_tensor(
                            ysq[:, 128 + 64 * cl:128 + 64 * cl + 64],
                            ysq[:, ts(cl, 64)], ysq[:, ts(cl, 64)], op=OP.mult)
                        # snapshot g/b for this chunk before update
                        nc.gpsimd.tensor_copy(gbsnap[:, ts(cl, 2)], gbc)
                        # scan updates: m = mom*m + s ; W = (1-dec)*W + m
                        momc = mdrep[:, c:c + 1]
                        decc = mdrep[:, NCH + c:NCH + c + 1]
                        nc.vector.scalar_tensor_tensor(m1, in0=m1, scalar=momc,
                                                       in1=pdw1, op0=OP.mult,
                                                       op1=OP.add)
                        nc.vector.scalar_tensor_tensor(w1c, in0=w1c,
                                                       scalar=decc, in1=m1,
                                                       op0=OP.mult, op1=OP.add)
                        nc.gpsimd.tensor_copy(w1cb, w1c)
                        nc.vector.scalar_tensor_tensor(m2, in0=m2, scalar=momc,
                                                       in1=pdw2, op0=OP.mult,
                                                       op1=OP.add)
                        nc.vector.scalar_tensor_tensor(w2c, in0=w2c,
                                                       scalar=decc, in1=m2,
                                                       op0=OP.mult, op1=OP.add)
                        nc.gpsimd.tensor_copy(w2cb, w2c)
                        sgbc = sgb.rearrange("p (a b) -> p a b", a=2)[:, :, cl]
                        nc.vector.scalar_tensor_tensor(mgb, in0=mgb,
                                                       scalar=momc, in1=sgbc,
                                                       op0=OP.mult, op1=OP.add)
                        nc.vector.scalar_tensor_tensor(gbc, in0=gbc,
                                                       scalar=decc, in1=mgb,
                                                       op0=OP.mult, op1=OP.add)

                    # ---- retrieve LN + gate + comb for both chunks ----
                    pst = psR.tile([128, 256], f32, tag="r")
                    nc.tensor.matmul(pst, onescol, ysq, start=True, stop=True)
                    nc.vector.tensor_scalar(murstd[0:1, 0:128], pst[0:1, 0:128],
                                            1.0 / DH, None, op0=OP.mult)
                    mu2 = wk.tile([1, 128], f32, tag="mu2")
                    nc.vector.tensor_tensor(mu2, murstd[0:1, 0:128],
                                            murstd[0:1, 0:128], op=OP.mult)
                    varr = wk.tile([1, 128], f32, tag="varr")
                    nc.vector.scalar_tensor_tensor(varr, in0=pst[0:1, 128:256],
                                                   scalar=1.0 / DH, in1=mu2,
                                                   op0=OP.mult,
                                                   op1=OP.subtract)
                    nc.scalar.activation(varr, varr, AF.Sqrt, bias=eps1a)
                    with nc.allow_low_precision(reason="f32r rstd for bcast mm"):
                        nc.vector.reciprocal(murstd[0:1, 128:256], varr)
                    pbc = psR.tile([128, 256], f32, tag="r")
                    nc.tensor.matmul(pbc, ones1, murstd, start=True, stop=True)
                    xhT = wk.tile([128, 128], f32, tag="xhT")
                    nc.vector.tensor_tensor(xhT, ysq[:, 0:128], pbc[:, 0:128],
                                            op=OP.subtract)
                    nc.vector.tensor_tensor(xhT, xhT, pbc[:, 128:256],
                                            op=OP.mult)
                    outTb = wk.tile([128, 128], bf16, tag="outTb")
                    for cl in range(2):
                        nc.vector.tensor_scalar(outTb[:, ts(cl, 64)],
                                                xhT[:, ts(cl, 64)],
                                                gbsnap[:, 2 * cl:2 * cl + 1],
                                                gbsnap[:, 2 * cl + 1:2 * cl + 2],
                                                op0=OP.mult, op1=OP.add)
                    pcomb = psA.tile([128, DIM], f32, tag="a")
                    nc.tensor.matmul(pcomb, outTb, wcombb, start=True,
                                     stop=True)
                    outst = wk.tile([128, DIM], f32, tag="outst")
                    nc.vector.tensor_scalar(outst, pcomb, gatec[:, t:t + 1],
                                            None, op0=OP.mult)
                    nc.sync.dma_start(out_d[ts(t, 128), :], outst)

    nc.compile()
    return nc, dt_in


def _prep_inputs(inputs):
    """Fold norms into weights; build the 8 per-core input dicts."""
    x = np.asarray(inputs["x"], np.float32)
    g_sto = np.asarray(inputs["g_sto"], np.float32)
    g_ret = np.asarray(inputs["g_ret"], np.float32)
    Wq = np.asarray(inputs["Wq"], np.float32)
    Wk = np.asarray(inputs["Wk"], np.float32)
    Wv = np.asarray(inputs["Wv"], np.float32)
    W_lr = np.asarray(inputs["W_lr"], np.float32)
    b_lr = np.asarray(inputs["b_lr"], np.float32)
    W_mom = np.asarray(inputs["W_mom"], np.float32)
    b_mom = np.asarray(inputs["b_mom"], np.float32)
    W_dec = np.asarray(inputs["W_dec"], np.float32)
    b_dec = np.asarray(inputs["b_dec"], np.float32)
    W_gate = np.asarray(inputs["W_gate"], np.float32)
    b_gate = np.asarray(inputs["b_gate"], np.float32)
    W_comb = np.asarray(inputs["W_comb"], np.float32)
    mw1 = np.asarray(inputs["mw1"], np.float32)
    mw2 = np.asarray(inputs["mw2"], np.float32)
    mg = np.asarray(inputs["mg"], np.float32)
    mb = np.asarray(inputs["mb"], np.float32)

    gs = g_sto[:, None]
    gr = g_ret[:, None]

    p = np.arange(128)
    mask2 = np.stack([(p < 64), (p >= 64)], 1).astype(np.float32)
    consts = dict(
        identf=np.eye(128, dtype=np.float32),
        identb=np.eye(128, dtype=np.float32),
        ones1=np.ones((1, 128), np.float32),
        onescol=np.concatenate([np.ones((128, 1), np.float32),
                                np.zeros((128, 127), np.float32)], 1),
        mask2=mask2,
        maskmean=mask2 / CHUNK,
    )

    in_maps = []
    for core in range(8):
        b, h = divmod(core, 4)
        projw = np.zeros((DIM, PCOLS), np.float32)
        projw[:, 0:128] = gs * Wk[:, ts(h, DH)]
        projw[:, 128:256] = gs * Wv[:, ts(h, DH)]
        projw[:, 256:384] = gr * Wq[:, ts(h, DH)]
        projw[:, 384] = g_sto * W_lr[:, h]
        projw[:, 385] = g_ret * W_gate[:, h]
        projw[:, 386] = g_sto * W_mom[:, h]
        projw[:, 387] = g_sto * W_dec[:, h]
        w1 = mw1[h]                          # [128, 512]
        w2 = mw2[h]                          # [512, 128]
        w2n = w2.reshape(4, 128, 128).transpose(1, 0, 2).copy()  # [p, j, dh]
        m = dict(
            x=x[b],
            projw=projw.reshape(4, 128, PCOLS).copy(),
            w1f=w1, w1b=w1,
            w2nf=w2n, w2nb=w2n,
            w2tb=w2.T.copy(),
            wcombb=W_comb[ts(h, DH), :].copy(),
            gbrow=np.concatenate([mg[h], mb[h]])[None, :],
            gbcol=np.stack([mg[h], mb[h]], 1),
            biaslg=np.array([[b_lr[h], b_gate[h]]], np.float32),
            biasmd=np.concatenate([np.full(NCH, b_mom[h], np.float32),
                                   np.full(NCH, b_dec[h], np.float32)])[None, :],
            **consts,
        )
        in_maps.append(m)
    return in_maps


def _cast_map(m, dt_in):
    import ml_dtypes
    out = {}
    for k, v in m.items():
        _, dt = dt_in[k]
        if dt == bf16:
            out[k] = np.asarray(v).astype(ml_dtypes.bfloat16)
        else:
            out[k] = np.asarray(v, np.float32)
    return out


def kernel(**inputs):
    if "nc" not in _CACHE:
        _CACHE["nc"], _CACHE["dt_in"] = _build()
    nc, dt_in = _CACHE["nc"], _CACHE["dt_in"]
    in_maps = [_cast_map(m, dt_in) for m in _prep_inputs(inputs)]
    res = bass_utils.run_bass_kernel_spmd(nc, in_maps, core_ids=list(range(8)))
    _CACHE["last_results"] = res
    b_comb = np.asarray(inputs["b_comb"], np.float32)
    outs = []
    for b in range(B):
        acc = b_comb[None, :].astype(np.float32).repeat(N, 0)
        for h in range(HEADS):
            acc = acc + res.results[4 * b + h]["out"]
        outs.append(acc)
    return np.stack(outs, 0)
